# revision 7
# baseline (speedup 1.0000x reference)
"""Causal multi-head attention (B=4, N=4, L=1024, H=8, E=64) on 8 trn2 cores.

Sharding: the 16 (b, n) pairs are split 2-per-core (batch/head-group data
parallelism) -- each core runs the identical Bass program on its own slice,
no cross-core communication.

v2 vs baseline:
  - Inputs are converted to bf16 on the host: halves input DMA, makes the
    PE input transposes 1 cycle/row (vs 2 for fp32), and keeps QK/AV
    matmuls at full PE rate.
  - QK matmuls of the two heads of a head-pair use disjoint PE row groups
    (contract dim 64 at base partitions 0/64), so consecutive hh0/hh1
    matmuls execute CONCURRENTLY in the array (row tiling) -- ~2x QK.
  - Output is produced in [e, l]-major DRAM layout: the av result [65, 512]
    is normalized in place (reciprocal of the ones-row, PE-broadcast to 64
    partitions, one DVE multiply) and stored with 2KB-contiguous DMA lines.
    No PE output transposes; the host fixes the layout with np.transpose.

Per-core algorithm (inputs flattened to [2*1024, H*E] = [2048, 512] bf16):
  for each (b,n) pair:
    - DMA Q/K slabs in natural [l, (h,e)] layout, PE-transpose 128x128
      blocks (2 heads at a time) into Q^T/K^T [e, l] bf16 slabs.
    - DMA V into a padded [k, lt, h, 65] bf16 layout with a ones-column
      (the ones row of V~ makes the AV matmul emit softmax denominators
      for free).
    - per (head-pair, 512-query group) "super-unit": S^T_j = K_j Q^T for
      both heads via row-tiled bf16 matmuls, exp on ACT engine (scale=1/8
      folded in; no max-subtraction needed), causal mask = elementwise
      multiply of diagonal blocks with a 0/1 constant on gpsimd, AV
      accumulation (V~^T @ expS^T) in PSUM over j, then the broadcast-
      normalize epilogue above.
"""

import sys

if "/opt/trn_rl_repo" not in sys.path:
    sys.path.insert(0, "/opt/trn_rl_repo")

import numpy as np

_CACHE = {}

B, N, L, H, E = 4, 4, 1024, 8, 64
CORES = 8
PAIRS = (B * N) // CORES  # (b,n) pairs per core
ROWS = PAIRS * L  # dram rows per core
HE = H * E
LT = L // 128  # 128-row l-tiles per pair
ORows = PAIRS * H * E  # output rows per core ([e, l]-major)


def _build(reps=1):
    key = ("nc", reps)
    if key in _CACHE:
        return _CACHE[key]

    import ml_dtypes
    import concourse.bass as bass
    import concourse.tile as tile
    from concourse import bacc, mybir

    f32 = mybir.dt.float32
    f32r = mybir.dt.float32r
    bf16 = mybir.dt.bfloat16
    np_bf16 = ml_dtypes.bfloat16
    AF = mybir.ActivationFunctionType

    nc = bacc.Bacc("TRN2", target_bir_lowering=False, debug=False, num_devices=CORES)
    qd = nc.dram_tensor("queries", [ROWS, HE], bf16, kind="ExternalInput").ap()
    kd = nc.dram_tensor("keys", [ROWS, HE], bf16, kind="ExternalInput").ap()
    vd = nc.dram_tensor("values", [ROWS, HE], bf16, kind="ExternalInput").ap()
    od = nc.dram_tensor("out", [ORows, L], f32, kind="ExternalOutput").ap()

    # Triangle mask: mask_np[k, c] = 1.0 iff c >= k. Every diagonal S^T block
    # reduces to this after the fully-masked leading columns are excluded
    # from the AV accumulation region.
    cols = np.arange(128)[None, :]
    rows = np.arange(128)[:, None]
    mask_np = (cols >= rows).astype(np_bf16)
    maskd = nc.inline_tensor(mask_np, name="cmasks").ap()
    identd = nc.inline_tensor(np.eye(128, dtype=np_bf16), name="ident").ap()
    onesd = nc.inline_tensor(np.ones((128, 1), dtype=np_bf16), name="ones").ap()
    onesrd = nc.inline_tensor(np.ones((1, E), dtype=np.float32), name="onesr").ap()

    with tile.TileContext(nc) as tc:
        with (
            tc.tile_pool(name="const", bufs=1) as cpool,
            tc.tile_pool(name="load", bufs=8) as lpool,
            tc.tile_pool(name="qt", bufs=2) as qtpool,
            tc.tile_pool(name="kt", bufs=2) as ktpool,
            tc.tile_pool(name="vp", bufs=2) as vppool,
            tc.tile_pool(name="es", bufs=18) as espool,
            tc.tile_pool(name="o", bufs=4) as opool,
            tc.tile_pool(name="r", bufs=4) as rpool,
            tc.tile_pool(name="rb", bufs=4) as rbpool,
            tc.tile_pool(name="ps_s", bufs=2, space="PSUM") as pss,
            tc.tile_pool(name="ps_av", bufs=2, space="PSUM") as psav,
            tc.tile_pool(name="ps_t", bufs=2, space="PSUM") as pst,
        ):
            ident = cpool.tile([128, 128], bf16)
            nc.sync.dma_start(ident[:, :], identd[:, :])
            ones = cpool.tile([128, 1], bf16)
            nc.sync.dma_start(ones[:, :], onesd[:, :])
            onesr = cpool.tile([1, E], f32)
            nc.sync.dma_start(onesr[:, :], onesrd[:, :])
            masks = cpool.tile([128, 128], bf16)

            slabs = {}

            def alloc_slab(pair):
                qt = qtpool.tile([128, 4, L], bf16, tag="qt")
                kt = ktpool.tile([128, 4, L], bf16, tag="kt")
                vp = vppool.tile([128, LT, H, E + 1], bf16, tag="vp")
                nc.gpsimd.tensor_copy(
                    vp[:, :, :, E : E + 1],
                    ones.broadcast_to([128, LT, H, 1]),
                )
                slabs[pair] = (qt, kt, vp)

            def emit_slab_qk(pair, lt, cold=False):
                qt, kt, _ = slabs[pair]
                r0 = pair * L + lt * 128
                tpool, ttag = (pss, "s") if cold else (pst, "tp")
                qload = lpool.tile([128, HE], bf16, tag="ld")
                nc.sync.dma_start(qload[:, :], qd[r0 : r0 + 128, :])
                tq = tpool.tile([128, 4, 128], bf16, tag=ttag)
                for pr in range(4):
                    nc.tensor.transpose(
                        tq[:, pr, :], qload[:, pr * 128 : (pr + 1) * 128], ident[:, :]
                    )
                nc.vector.tensor_copy(qt[:, :, lt * 128 : (lt + 1) * 128], tq[:, :, :])

                kload = lpool.tile([128, HE], bf16, tag="ld")
                nc.sync.dma_start(kload[:, :], kd[r0 : r0 + 128, :])
                tk = tpool.tile([128, 4, 128], bf16, tag=ttag)
                for pr in range(4):
                    nc.tensor.transpose(
                        tk[:, pr, :], kload[:, pr * 128 : (pr + 1) * 128], ident[:, :]
                    )
                nc.vector.tensor_copy(kt[:, :, lt * 128 : (lt + 1) * 128], tk[:, :, :])

            def emit_slab_v(pair, lt):
                _, _, vp = slabs[pair]
                r0 = pair * L + lt * 128
                vload = lpool.tile([128, HE], bf16, tag="ld")
                nc.sync.dma_start(vload[:, :], vd[r0 : r0 + 128, :])
                nc.gpsimd.tensor_copy(
                    vp[:, lt, :, 0:E],
                    vload.rearrange("p (h e) -> p h e", e=E),
                )

            def unit_phase1(pair, hp, qg):
                qt, kt, vp = slabs[pair]
                jn = 4 * qg + 4  # causal: only j-tiles <= query group
                # Phase 1: all QK matmuls + exp + mask. hh0/hh1 matmuls are
                # interleaved: their contract dims live at PE row groups
                # 0-63 / 64-127, so consecutive instructions execute
                # concurrently in the array.
                ess = []
                for jp in range(jn // 2):
                    # Both j's of a diagonal pair share a leading fully-masked
                    # column range of >= 128*(2*jp-4*qg) columns; skip it in
                    # the QK matmuls AND the exp, since the AV matmuls never
                    # read it.
                    tp0 = 2 * jp - 4 * qg
                    sk = 128 * tp0 if tp0 > 0 else 0
                    s0 = pss.tile([128, 1024], f32, tag="s")
                    s1 = pss.tile([128, 1024], f32, tag="s")
                    ss = [s0, s1]
                    for half in range(2):
                        j = 2 * jp + half
                        for hh in range(2):
                            lhsT = kt[64 * hh : 64 * hh + 64, hp, j * 128 : (j + 1) * 128]
                            rhs = qt[
                                64 * hh : 64 * hh + 64,
                                hp,
                                qg * 512 + sk : (qg + 1) * 512,
                            ]
                            nc.tensor.matmul(
                                ss[hh][:, half * 512 + sk : (half + 1) * 512],
                                lhsT,
                                rhs,
                                start=True,
                                stop=True,
                            )
                    pes = []
                    for hh in range(2):
                        es = espool.tile([128, 1024], bf16, tag="es")
                        sv = ss[hh].rearrange("p (u c) -> p u c", u=2)[:, :, sk:512]
                        ev = es.rearrange("p (u c) -> p u c", u=2)[:, :, sk:512]
                        nc.scalar.activation(ev, sv, AF.Exp, scale=0.125)
                        pes.append(es)
                    t0 = 2 * jp - 4 * qg
                    if t0 >= 0:
                        # Diagonal pair: only the [128,127] triangles (at
                        # column offsets 128*t, the two halves 640 columns
                        # apart) need masking -- the fully masked leading
                        # columns are excluded from the AV accumulation
                        # region instead. One 2-piece strided op per head.
                        c0 = 128 * t0

                        def tri(ap, off=c0):
                            return bass.AP(
                                ap.tensor,
                                ap.offset + off,
                                [list(ap.ap[0]), [640, 2], [1, 127]],
                            )

                        mb = bass.AP(
                            masks.tensor,
                            masks.offset,
                            [list(masks.ap[0]), [0, 2], [1, 127]],
                        )
                        for hh in range(2):
                            nc.gpsimd.tensor_mul(tri(pes[hh]), tri(pes[hh]), mb)
                    ess.append(pes)
                return ess

            def unit_phase2(pair, hp, qg, ess):
                _, _, vp = slabs[pair]
                jn = 4 * qg + 4
                av0 = psav.tile([E + 1, 512], f32, tag="av")
                av1 = psav.tile([E + 1, 512], f32, tag="av")
                avs = [av0, av1]
                for jp in range(jn // 2):
                    for half in range(2):
                        j = 2 * jp + half
                        t = j - 4 * qg
                        # Diagonal blocks with t>=1: their leading 128*t
                        # columns are fully causally masked, so restrict the
                        # matmul to the unmasked column range.
                        c0 = 128 * t if t > 0 else 0
                        for hh in range(2):
                            nc.tensor.matmul(
                                avs[hh][:, c0:512],
                                vp[:, j, 2 * hp + hh, :],
                                ess[jp][hh][:, half * 512 + c0 : (half + 1) * 512],
                                start=(j == 0),
                                stop=(j == jn - 1),
                                skip_group_check=True,
                            )
                return avs

            def unit_epilogue(pair, hp, qg, avs):
                # normalize in [e, q] orientation and store [e, l]-major:
                # reciprocal of the denominators row, PE-broadcast it across
                # the 64 feature partitions, one multiply, contiguous DMA.
                for hh in range(2):
                    av = avs[hh]
                    r = rpool.tile([1, 512], f32, tag="r")
                    nc.vector.reciprocal(r[:, :], av[E : E + 1, :])
                    rb = rbpool.tile([E, 512], f32, tag="rb")
                    rsrc = bass.AP(
                        r.tensor, r.offset, [list(r.ap[0]), [0, E], [1, 512]]
                    )
                    nc.sync.dma_start(rb[:, :], rsrc)
                    o = opool.tile([E, 512], f32, tag="o")
                    nc.vector.tensor_mul(o[:, :], av[0:E, :], rb[:, :])
                    base = (pair * H + 2 * hp + hh) * E
                    nc.sync.dma_start(
                        od[base : base + E, qg * 512 : (qg + 1) * 512], o[:, :]
                    )

            pending = [None]

            def compute_unit(pair, hp, qg):
                # Software-pipelined: the previous super-unit's epilogue is
                # emitted between this unit's QK phase and AV phase.
                ess = unit_phase1(pair, hp, qg)
                if pending[0] is not None:
                    unit_epilogue(*pending[0])
                avs = unit_phase2(pair, hp, qg, ess)
                pending[0] = (pair, hp, qg, avs)

            import contextlib

            loop_ctx = tc.For_i(0, reps) if reps > 1 else contextlib.nullcontext()
            # Schedule: qg0 super-units only need l-tiles 0-3, so run all of
            # them first (halves the cold start); spread the remaining slab
            # loads and the next pair's slab across the compute units.
            with loop_ctx:
                alloc_slab(0)
                for lt in range(4):
                    emit_slab_qk(0, lt, cold=True)
                nc.sync.dma_start(masks[:, :], maskd[:, :])
                for lt in range(4):
                    emit_slab_v(0, lt)

                for u in range(4):  # pair 0, qg0
                    emit_slab_qk(0, 4 + u)
                    emit_slab_v(0, 4 + u)
                    compute_unit(0, u, 0)
                for u in range(4):  # pair 0, qg1 -- interleave pair-1 slab
                    if u == 0:
                        alloc_slab(1)
                    emit_slab_qk(1, u)
                    emit_slab_v(1, u)
                    compute_unit(0, u, 1)
                for u in range(4):  # pair 1, qg0 -- rest of slab 1
                    emit_slab_qk(1, 4 + u)
                    emit_slab_v(1, 4 + u)
                    compute_unit(1, u, 0)
                for u in range(4):  # pair 1, qg1
                    compute_unit(1, u, 1)
                unit_epilogue(*pending[0])
                pending[0] = None

    nc.compile()
    _CACHE[key] = nc
    if reps == 1:
        _CACHE["nc"] = nc
    return nc


def _shard(x):
    # [B, N, L, H, E] -> per-core [ROWS, HE] bf16 slices
    import ml_dtypes

    flat = (
        np.asarray(x)
        .astype(ml_dtypes.bfloat16)
        .reshape(B * N, L, HE)
    )
    return [
        np.ascontiguousarray(flat[c * PAIRS : (c + 1) * PAIRS].reshape(ROWS, HE))
        for c in range(CORES)
    ]


def kernel(queries, keys, values):
    from concourse.bass_utils import run_bass_kernel_spmd

    nc = _build()
    qs, ks, vs = _shard(queries), _shard(keys), _shard(values)
    in_maps = [
        {"queries": qs[c], "keys": ks[c], "values": vs[c]} for c in range(CORES)
    ]
    res = run_bass_kernel_spmd(nc, in_maps, core_ids=list(range(CORES)))
    # device output is [PAIRS, H, E, L]; transpose to [PAIRS, L, H, E]
    out = np.concatenate(
        [
            res.results[c]["out"]
            .reshape(PAIRS, H, E, L)
            .transpose(0, 3, 1, 2)
            for c in range(CORES)
        ]
    )
    return np.ascontiguousarray(out.reshape(B, N, L, H, E)).astype(np.float32)
